# revision 28
# baseline (speedup 1.0000x reference)
"""Linear attention (elu+1 feature map) Bass/Tile kernel for Trainium2.

Full inputs: queries/keys/values [N=8, L/S=8192, H=8, D=64] fp32.
Sharding: data-parallel over N across the 8 NeuronCores (batch i -> core i).
Inputs are cast to bf16 on the host (round-to-nearest); output is bf16 and
upcast on the host.  All PSUM accumulation is fp32.

Math per (n, h):
  Q' = elu(Q)+1, K' = elu(K)+1
  KV[d, v] = sum_s K'[s, d] V[s, v]     (the /S, *S in the reference cancel
  Ksum[d]  = sum_s K'[s, d]              exactly: S = 2^13)
  out[l, v] = (Q'[l, :] @ KV[:, v]) / (Q'[l, :] @ Ksum)   (EPS dropped:
  denom ~ O(5000), the reference's 1e-6 is far below bf16 noise)

Feature map fm(x) = max(x,0) + exp(min(x,0)), two engine-balanced variants:
  chain C: m = min(x,0) [DVE 4x, exact]; e = Exp(m) -> fp32 [ACT, table is
    only accurate with fp32 out]; fm = STT max(x,0)+e [DVE 1x]
  chain G: m, r = min/max(x,0) [DVE 4x]; e = Exp(m) -> bf16 [ACT, ~2ulp];
    fm = r + e on GPSIMD [otherwise idle engine]
Tiles alternate variants to balance DVE vs GPSIMD occupancy.

Phase 1 (stream K, V in 512-row chunks): per head PAIR one matmul
  lhsT=K'_pair [128,128] rhs=V_pair accumulated into psum [128, 0:128], plus
  an N=1 ones-matmul into col 128 of the same bank for Ksum (start=False
  always: the V-chain's start clears the bank's has_written bits, so the
  first ones-matmul overwrites per-element and accumulates after).
Phase 1.5: copy KV block-diagonals to bf16 w2[j] [128,128] (cross blocks
  zeroed), scatter Ksum into block rhs kb[g] [128, 8].
Phase 2 (stream Q in 2048-row mega-chunks): per head-group g an xbar
  DMA-transpose loads Q^T [128 hd, 2048 l] straight from DRAM (measured
  bit-exact); feature-map; then per 512-l block: 16 N=8 denominator matmuls
  into one [128, 32] psum, one reciprocal, and per 128-l sub 4 output
  matmuls into a [128, 512] psum followed by a single broadcast
  tensor_tensor multiply into the bf16 out tile.
"""

import functools
import sys

sys.path.insert(0, "/opt/trn_rl_repo")

import numpy as np
import ml_dtypes

import concourse.bass as bass
import concourse.mybir as mybir
import concourse.tile as tile
from concourse import bacc
from concourse.bass_utils import run_bass_kernel_spmd

N, L, S, H, D = 8, 8192, 8192, 8, 64
P = 128
HD = H * D
BF16 = mybir.dt.bfloat16
FP32 = mybir.dt.float32
AF = mybir.ActivationFunctionType
OP = mybir.AluOpType
KC = 512    # K/V chunk rows
QC = 2048   # Q mega-chunk rows


def _fm(nc, pool, x, out, shape, tag, gpsimd):
    """out = elu(x)+1 = max(x,0) + exp(min(x,0)), bf16 in/out.

    gpsimd variant (chain G): min/max on DVE (tensor_scalar), Exp on ACT
    (bf16 out, ~2ulp table), add on the otherwise-idle GPSIMD.
    DVE variant (chain C): min on DVE, Exp on ACT with fp32 out (the Exp
    table is only accurate with fp32 out), one STT on DVE.
    """
    m = pool.tile(shape, BF16, name=f"fm_m_{tag}", tag="fm_m")
    nc.vector.tensor_scalar_min(m, x, 0.0)
    if gpsimd:
        e = pool.tile(shape, BF16, name=f"fm_eb_{tag}", tag="fm_eb")
        nc.scalar.activation(e, m, AF.Exp)
        r = pool.tile(shape, BF16, name=f"fm_r_{tag}", tag="fm_r")
        nc.vector.tensor_scalar_max(r, x, 0.0)
        nc.gpsimd.tensor_add(out, r, e)
    else:
        e = pool.tile(shape, FP32, name=f"fm_ef_{tag}", tag="fm_ef")
        nc.scalar.activation(e, m, AF.Exp)
        nc.vector.scalar_tensor_tensor(
            out, in0=x, scalar=0.0, in1=e, op0=OP.max, op1=OP.add
        )


def build_kernel(L_=L, S_=S):
    nc = bacc.Bacc(trn_type="TRN2")
    q_d = nc.dram_tensor("queries", [L_, HD], BF16, kind="ExternalInput")
    k_d = nc.dram_tensor("keys", [S_, HD], BF16, kind="ExternalInput")
    # values are host-packed [S, 4 pairs, 130]: cols 0:128 = the pair's V
    # columns, col 128 = 1.0 (folds the Ksum ones-column into the KV matmul),
    # col 129 = alignment pad
    v_d = nc.dram_tensor("values", [S_, 4 * 130], BF16, kind="ExternalInput")
    o_d = nc.dram_tensor("out", [L_, HD], BF16, kind="ExternalOutput")

    n_kc = S_ // KC
    n_qc = L_ // QC

    with tile.TileContext(nc) as tc:
        with (
            tc.tile_pool(name="kdma", bufs=3) as kdma,
            tc.tile_pool(name="vdma", bufs=3) as vdma,
            tc.tile_pool(name="fmk", bufs=2) as fmk,
            tc.tile_pool(name="wp", bufs=1) as wp,
            tc.tile_pool(name="qdma", bufs=4) as qdma,
            tc.tile_pool(name="fmq", bufs=2) as fmq,
            tc.tile_pool(name="zp", bufs=2) as zp,
            tc.tile_pool(name="outp", bufs=3) as outp,
        ):
            w2 = [wp.tile([P, P], BF16, name=f"w2_{j}", tag=f"w2_{j}") for j in range(4)]
            kb = [wp.tile([P, 8], BF16, name=f"kb_{j}", tag=f"kb_{j}") for j in range(4)]

            # Q-prep (xbar-transpose load + feature map) is emitted just-in-
            # time, one mega-chunk ahead of consumption: an early-emitted
            # prefetch whose SBUF slot isn't free yet parks a long semaphore
            # wait at the head of the in-order Sync DMA queue and blocks every
            # K/V/out DMA behind it (measured 40us stalls).
            qp_store = {}

            def emit_qprep(qc):
                l0 = qc * QC
                tiles = []
                for g in range(4):
                    qt = qdma.tile([P, QC], BF16, name=f"qt{g}", tag="qt")
                    nc.sync.dma_start_transpose(
                        qt, q_d[l0 : l0 + QC, g * P : (g + 1) * P]
                    )
                    qpg = fmq.tile([P, QC], BF16, name=f"qp{g}", tag=f"qp{g}")
                    # pure DVE/ACT chain: concurrent GPSIMD tensor ops
                    # port-starve DVE 2-port modes (measured 684ns -> 4.5us
                    # tensor_scalar); GPSIMD is used only in phase 1 where
                    # DVE has slack
                    _fm(nc, fmq, qt, qpg, [P, QC], f"q{qc}_{g}", gpsimd=False)
                    tiles.append(qpg)
                qp_store[qc] = tiles

            # ---- Phase 1: KV + Ksum accumulation ----
            with tc.tile_pool(name="kvps", bufs=1, space="PSUM") as kvps:
                kv_ps = [
                    kvps.tile([P, 129], FP32, name=f"kv{j}", tag=f"kv{j}")
                    for j in range(4)
                ]
                for cc in range(n_kc):
                    r0 = cc * KC
                    ktile = kdma.tile([P, 4 * HD], BF16, name="ktile", tag="ktile")
                    nc.sync.dma_start(
                        ktile.rearrange("p (c f) -> p c f", c=4),
                        k_d[r0 : r0 + KC, :].rearrange("(c p) f -> p c f", p=P),
                    )
                    vtile = vdma.tile([P, 4, 4, 130], BF16, name="vtile", tag="vtile")
                    nc.sync.dma_start(
                        vtile,
                        v_d[r0 : r0 + KC, :].rearrange(
                            "(c p) (j e) -> p c j e", p=P, j=4
                        ),
                    )
                    kp = fmk.tile([P, 4 * HD], BF16, name="kp", tag="kp")
                    _fm(nc, fmk, ktile, kp, [P, 4 * HD], f"k{cc}", gpsimd=False)
                    first = cc == 0
                    last = cc == n_kc - 1
                    for sub in range(4):
                        for j in range(4):
                            nc.tensor.matmul(
                                kv_ps[j],
                                lhsT=kp[:, (sub * 4 + j) * P : (sub * 4 + j + 1) * P],
                                rhs=vtile[:, sub, j, 0:129],
                                start=(first and sub == 0),
                                stop=(last and sub == 3),
                            )
                    if cc == n_kc - 5:
                        emit_qprep(0)

                # ---- Phase 1.5: block-diag KV weights (bf16) + Ksum blocks ----
                for j in range(4):
                    nc.vector.memset(w2[j], 0.0)
                    nc.vector.tensor_copy(w2[j][0:64, 0:64], kv_ps[j][0:64, 0:64])
                    nc.vector.tensor_copy(
                        w2[j][64:128, 64:128], kv_ps[j][64:128, 64:128]
                    )
                    nc.vector.memset(kb[j], 0.0)
                    nc.vector.tensor_copy(
                        kb[j][0:64, 2 * j : 2 * j + 1], kv_ps[j][0:64, P : P + 1]
                    )
                    nc.vector.tensor_copy(
                        kb[j][64:128, 2 * j + 1 : 2 * j + 2],
                        kv_ps[j][64:128, P : P + 1],
                    )

            # ---- Phase 2: stream Q ----
            with (
                tc.tile_pool(name="ops", bufs=4, space="PSUM") as opsp,
                tc.tile_pool(name="dps", bufs=2, space="PSUM") as dpsp,
            ):
                for qc in range(n_qc):
                    l0 = qc * QC
                    if qc + 1 < n_qc:
                        emit_qprep(qc + 1)
                    qp = qp_store.pop(qc)

                    for blk in range(QC // KC):  # 4 blocks of 512 l-rows
                        den_ps = dpsp.tile([P, 32], FP32, name="den", tag="den")
                        for sub4 in range(4):
                            sub = blk * 4 + sub4
                            for g in range(4):
                                nc.tensor.matmul(
                                    den_ps[:, sub4 * 8 : (sub4 + 1) * 8],
                                    lhsT=qp[g][:, sub * P : (sub + 1) * P],
                                    rhs=kb[g],
                                    start=(g == 0),
                                    stop=(g == 3),
                                    skip_group_check=True,
                                )
                        zr = zp.tile([P, 32], FP32, name="zr", tag="zr")
                        nc.vector.reciprocal(zr, den_ps)
                        otile = outp.tile([P, 4, HD], BF16, name="otile", tag="otile")
                        for sub4 in range(4):
                            sub = blk * 4 + sub4
                            out_ps = opsp.tile([P, HD], FP32, name="op", tag="op")
                            for g in range(4):
                                nc.tensor.matmul(
                                    out_ps[:, g * P : (g + 1) * P],
                                    lhsT=qp[g][:, sub * P : (sub + 1) * P],
                                    rhs=w2[g],
                                    start=True,
                                    stop=True,
                                )
                            zb = (
                                zr[:, sub4 * 8 : (sub4 + 1) * 8]
                                .unsqueeze(2)
                                .to_broadcast([P, 8, 64])
                            )
                            nc.vector.tensor_mul(
                                otile[:, sub4, :].rearrange("p (h v) -> p h v", h=H),
                                out_ps.rearrange("p (h v) -> p h v", h=H),
                                zb,
                            )
                        nc.sync.dma_start(
                            o_d[l0 + blk * KC : l0 + (blk + 1) * KC, :].rearrange(
                                "(c p) f -> p c f", p=P
                            ),
                            otile,
                        )
    nc.compile()
    return nc


@functools.lru_cache(maxsize=None)
def _cached_nc(L_, S_):
    return build_kernel(L_, S_)


def _to_bf16(x: np.ndarray) -> np.ndarray:
    """fp32 -> bf16 with round-to-nearest-even (vectorized, no ml_dtypes cast)."""
    u = np.ascontiguousarray(x, np.float32).view(np.uint32)
    r = (u + np.uint32(0x7FFF) + ((u >> np.uint32(16)) & np.uint32(1))) >> np.uint32(16)
    return r.astype(np.uint16).view(ml_dtypes.bfloat16)


def _from_bf16(x: np.ndarray) -> np.ndarray:
    u = np.ascontiguousarray(x).view(np.uint16).astype(np.uint32) << np.uint32(16)
    return u.view(np.float32)


def _pack_values(values: np.ndarray) -> np.ndarray:
    """[N, S, HD] fp32 -> bf16 [N, S, 4*130] with a ones column per head pair."""
    n, s_, hd = values.shape
    v = _to_bf16(values).view(np.uint16)
    out = np.zeros((n, s_, 4, 130), np.uint16)
    out[..., 128] = np.uint16(0x3F80)  # 1.0 in bf16
    out[..., 0:128] = v.reshape(n, s_, 4, 128)
    return out.reshape(n, s_, 4 * 130).view(ml_dtypes.bfloat16)


def kernel(queries: np.ndarray, keys: np.ndarray, values: np.ndarray) -> np.ndarray:
    n, l_, h, d = queries.shape
    s_ = keys.shape[1]
    hd = h * d
    q = _to_bf16(np.asarray(queries, np.float32).reshape(n, l_, hd))
    k = _to_bf16(np.asarray(keys, np.float32).reshape(n, s_, hd))
    v = _pack_values(np.asarray(values, np.float32).reshape(n, s_, hd))
    nc = _cached_nc(l_, s_)
    in_maps = [{"queries": q[i], "keys": k[i], "values": v[i]} for i in range(n)]
    res = run_bass_kernel_spmd(nc, in_maps, core_ids=list(range(n)))
    out = np.empty((n, l_, h, d), np.float32)
    for i in range(n):
        out[i] = _from_bf16(res.results[i]["out"]).reshape(l_, h, d)
    return out


if __name__ == "__main__":
    nc = build_kernel()
    print("build ok")


# revision 32
# speedup vs baseline: 1.1309x; 1.1309x over previous
"""Linear attention (elu+1 feature map) Bass/Tile kernel for Trainium2.

Full inputs: queries/keys/values [N=8, L/S=8192, H=8, D=64] fp32.
Sharding: data-parallel over N across the 8 NeuronCores (batch i -> core i).
Inputs are cast to bf16 on the host (round-to-nearest); output is bf16 and
upcast on the host.  All PSUM accumulation is fp32.

Math per (n, h):
  Q' = elu(Q)+1, K' = elu(K)+1
  KV[d, v] = sum_s K'[s, d] V[s, v]     (the /S, *S in the reference cancel
  Ksum[d]  = sum_s K'[s, d]              exactly: S = 2^13)
  out[l, v] = (Q'[l, :] @ KV[:, v]) / (Q'[l, :] @ Ksum)   (EPS dropped:
  denom ~ O(5000), the reference's 1e-6 is far below bf16 noise)

Feature map fm(x) = max(x,0) + exp(min(x,0)), two engine-balanced variants:
  chain C: m = min(x,0) [DVE 4x, exact]; e = Exp(m) -> fp32 [ACT, table is
    only accurate with fp32 out]; fm = STT max(x,0)+e [DVE 1x]
  chain G: m, r = min/max(x,0) [DVE 4x]; e = Exp(m) -> bf16 [ACT, ~2ulp];
    fm = r + e on GPSIMD [otherwise idle engine]
Tiles alternate variants to balance DVE vs GPSIMD occupancy.

Phase 1 (stream K, V in 512-row chunks): per head PAIR one matmul
  lhsT=K'_pair [128,128] rhs=V_pair accumulated into psum [128, 0:128], plus
  an N=1 ones-matmul into col 128 of the same bank for Ksum (start=False
  always: the V-chain's start clears the bank's has_written bits, so the
  first ones-matmul overwrites per-element and accumulates after).
Phase 1.5: copy KV block-diagonals to bf16 w2[j] [128,128] (cross blocks
  zeroed), scatter Ksum into block rhs kb[g] [128, 8].
Phase 2 (stream Q in 2048-row mega-chunks): per head-group g an xbar
  DMA-transpose loads Q^T [128 hd, 2048 l] straight from DRAM (measured
  bit-exact); feature-map; then per 512-l block: 16 N=8 denominator matmuls
  into one [128, 32] psum, one reciprocal, and per 128-l sub 4 output
  matmuls into a [128, 512] psum followed by a single broadcast
  tensor_tensor multiply into the bf16 out tile.
"""

import functools
import sys

sys.path.insert(0, "/opt/trn_rl_repo")

import numpy as np
import ml_dtypes

import concourse.bass as bass
import concourse.mybir as mybir
import concourse.tile as tile
from concourse import bacc
from concourse.bass_utils import run_bass_kernel_spmd

N, L, S, H, D = 8, 8192, 8192, 8, 64
P = 128
HD = H * D
BF16 = mybir.dt.bfloat16
FP32 = mybir.dt.float32
AF = mybir.ActivationFunctionType
OP = mybir.AluOpType
KC = 512    # K/V chunk rows
QC = 2048   # Q mega-chunk rows


def _fm(nc, pool, x, out, shape, tag, gpsimd):
    """out = elu(x)+1 = max(x,0) + exp(min(x,0)), bf16 in/out.

    gpsimd variant (chain G): min/max on DVE (tensor_scalar), Exp on ACT
    (bf16 out, ~2ulp table), add on the otherwise-idle GPSIMD.
    DVE variant (chain C): min on DVE, Exp on ACT with fp32 out (the Exp
    table is only accurate with fp32 out), one STT on DVE.
    """
    m = pool.tile(shape, BF16, name=f"fm_m_{tag}", tag="fm_m")
    nc.vector.tensor_scalar_min(m, x, 0.0)
    if gpsimd:
        e = pool.tile(shape, BF16, name=f"fm_eb_{tag}", tag="fm_eb")
        nc.scalar.activation(e, m, AF.Exp)
        r = pool.tile(shape, BF16, name=f"fm_r_{tag}", tag="fm_r")
        nc.vector.tensor_scalar_max(r, x, 0.0)
        nc.gpsimd.tensor_add(out, r, e)
    else:
        e = pool.tile(shape, FP32, name=f"fm_ef_{tag}", tag="fm_ef")
        nc.scalar.activation(e, m, AF.Exp)
        nc.vector.scalar_tensor_tensor(
            out, in0=x, scalar=0.0, in1=e, op0=OP.max, op1=OP.add
        )


def build_kernel(L_=L, S_=S):
    nc = bacc.Bacc(trn_type="TRN2")
    q_d = nc.dram_tensor("queries", [L_, HD], BF16, kind="ExternalInput")
    k_d = nc.dram_tensor("keys", [S_, HD], BF16, kind="ExternalInput")
    # values are host-packed [S, 4 pairs, 130]: cols 0:128 = the pair's V
    # columns, col 128 = 1.0 (folds the Ksum ones-column into the KV matmul),
    # col 129 = alignment pad
    v_d = nc.dram_tensor("values", [S_, 4 * 130], BF16, kind="ExternalInput")
    o_d = nc.dram_tensor("out", [L_, HD], BF16, kind="ExternalOutput")

    n_kc = S_ // KC
    n_qc = L_ // QC

    with tile.TileContext(nc) as tc:
        with (
            tc.tile_pool(name="kdma", bufs=4) as kdma,
            tc.tile_pool(name="vdma", bufs=4) as vdma,
            tc.tile_pool(name="fmk", bufs=3) as fmk,
            tc.tile_pool(name="wp", bufs=1) as wp,
            tc.tile_pool(name="qdma", bufs=4) as qdma,
            tc.tile_pool(name="fmq", bufs=2) as fmq,
            tc.tile_pool(name="zp", bufs=2) as zp,
            tc.tile_pool(name="outp", bufs=3) as outp,
        ):
            w2 = [wp.tile([P, P], BF16, name=f"w2_{j}", tag=f"w2_{j}") for j in range(4)]
            kb = [wp.tile([P, 8], BF16, name=f"kb_{j}", tag=f"kb_{j}") for j in range(4)]

            # Q-prep (xbar-transpose load + feature map) is emitted just-in-
            # time, one mega-chunk ahead of consumption: an early-emitted
            # prefetch whose SBUF slot isn't free yet parks a long semaphore
            # wait at the head of the in-order Sync DMA queue and blocks every
            # K/V/out DMA behind it (measured 40us stalls).
            qp_store = {}

            def emit_qprep(qc):
                l0 = qc * QC
                tiles = []
                for g in range(4):
                    qt = qdma.tile([P, QC], BF16, name=f"qt{g}", tag="qt")
                    nc.sync.dma_start_transpose(
                        qt, q_d[l0 : l0 + QC, g * P : (g + 1) * P]
                    )
                    qpg = fmq.tile([P, QC], BF16, name=f"qp{g}", tag=f"qp{g}")
                    # pure DVE/ACT chain: concurrent GPSIMD tensor ops
                    # port-starve DVE 2-port modes (measured 684ns -> 4.5us
                    # tensor_scalar); GPSIMD is used only in phase 1 where
                    # DVE has slack
                    _fm(nc, fmq, qt, qpg, [P, QC], f"q{qc}_{g}", gpsimd=False)
                    tiles.append(qpg)
                qp_store[qc] = tiles

            # ---- Phase 1: KV + Ksum accumulation ----
            with tc.tile_pool(name="kvps", bufs=1, space="PSUM") as kvps:
                kv_ps = [
                    kvps.tile([P, 129], FP32, name=f"kv{j}", tag=f"kv{j}")
                    for j in range(4)
                ]
                for cc in range(n_kc):
                    r0 = cc * KC
                    ktile = kdma.tile([P, 4 * HD], BF16, name="ktile", tag="ktile")
                    nc.sync.dma_start(
                        ktile.rearrange("p (c f) -> p c f", c=4),
                        k_d[r0 : r0 + KC, :].rearrange("(c p) f -> p c f", p=P),
                    )
                    vtile = vdma.tile([P, 4, 4, 130], BF16, name="vtile", tag="vtile")
                    nc.sync.dma_start(
                        vtile,
                        v_d[r0 : r0 + KC, :].rearrange(
                            "(c p) (j e) -> p c j e", p=P, j=4
                        ),
                    )
                    kp = fmk.tile([P, 4 * HD], BF16, name="kp", tag="kp")
                    _fm(nc, fmk, ktile, kp, [P, 4 * HD], f"k{cc}", gpsimd=(cc % 2 == 0))
                    first = cc == 0
                    last = cc == n_kc - 1
                    for sub in range(4):
                        for j in range(4):
                            nc.tensor.matmul(
                                kv_ps[j],
                                lhsT=kp[:, (sub * 4 + j) * P : (sub * 4 + j + 1) * P],
                                rhs=vtile[:, sub, j, 0:129],
                                start=(first and sub == 0),
                                stop=(last and sub == 3),
                            )
                    if cc == n_kc - 5:
                        emit_qprep(0)

                # ---- Phase 1.5: block-diag KV weights (bf16) + Ksum blocks ----
                for j in range(4):
                    nc.vector.memset(w2[j], 0.0)
                    nc.vector.tensor_copy(w2[j][0:64, 0:64], kv_ps[j][0:64, 0:64])
                    nc.vector.tensor_copy(
                        w2[j][64:128, 64:128], kv_ps[j][64:128, 64:128]
                    )
                    nc.vector.memset(kb[j], 0.0)
                    nc.vector.tensor_copy(
                        kb[j][0:64, 2 * j : 2 * j + 1], kv_ps[j][0:64, P : P + 1]
                    )
                    nc.vector.tensor_copy(
                        kb[j][64:128, 2 * j + 1 : 2 * j + 2],
                        kv_ps[j][64:128, P : P + 1],
                    )

            # ---- Phase 2: stream Q ----
            with (
                tc.tile_pool(name="ops", bufs=6, space="PSUM") as opsp,
                tc.tile_pool(name="dps", bufs=2, space="PSUM") as dpsp,
            ):
                for qc in range(n_qc):
                    l0 = qc * QC
                    if qc + 1 < n_qc:
                        emit_qprep(qc + 1)
                    qp = qp_store.pop(qc)

                    for blk in range(QC // KC):  # 4 blocks of 512 l-rows
                        den_ps = dpsp.tile([P, 32], FP32, name="den", tag="den")
                        for sub4 in range(4):
                            sub = blk * 4 + sub4
                            for g in range(4):
                                nc.tensor.matmul(
                                    den_ps[:, sub4 * 8 : (sub4 + 1) * 8],
                                    lhsT=qp[g][:, sub * P : (sub + 1) * P],
                                    rhs=kb[g],
                                    start=(g == 0),
                                    stop=(g == 3),
                                    skip_group_check=True,
                                )
                        zr = zp.tile([P, 32], FP32, name="zr", tag="zr")
                        nc.vector.reciprocal(zr, den_ps)
                        otile = outp.tile([P, 4, HD], BF16, name="otile", tag="otile")
                        for sub4 in range(4):
                            sub = blk * 4 + sub4
                            out_ps = opsp.tile([P, HD], FP32, name="op", tag="op")
                            for g in range(4):
                                nc.tensor.matmul(
                                    out_ps[:, g * P : (g + 1) * P],
                                    lhsT=qp[g][:, sub * P : (sub + 1) * P],
                                    rhs=w2[g],
                                    start=True,
                                    stop=True,
                                )
                            zb = (
                                zr[:, sub4 * 8 : (sub4 + 1) * 8]
                                .unsqueeze(2)
                                .to_broadcast([P, 8, 64])
                            )
                            nc.vector.tensor_mul(
                                otile[:, sub4, :].rearrange("p (h v) -> p h v", h=H),
                                out_ps.rearrange("p (h v) -> p h v", h=H),
                                zb,
                            )
                        nc.sync.dma_start(
                            o_d[l0 + blk * KC : l0 + (blk + 1) * KC, :].rearrange(
                                "(c p) f -> p c f", p=P
                            ),
                            otile,
                        )
    nc.compile()
    return nc


@functools.lru_cache(maxsize=None)
def _cached_nc(L_, S_):
    return build_kernel(L_, S_)


def _to_bf16(x: np.ndarray) -> np.ndarray:
    """fp32 -> bf16 with round-to-nearest-even (vectorized, no ml_dtypes cast)."""
    u = np.ascontiguousarray(x, np.float32).view(np.uint32)
    r = (u + np.uint32(0x7FFF) + ((u >> np.uint32(16)) & np.uint32(1))) >> np.uint32(16)
    return r.astype(np.uint16).view(ml_dtypes.bfloat16)


def _from_bf16(x: np.ndarray) -> np.ndarray:
    u = np.ascontiguousarray(x).view(np.uint16).astype(np.uint32) << np.uint32(16)
    return u.view(np.float32)


def _pack_values(values: np.ndarray) -> np.ndarray:
    """[N, S, HD] fp32 -> bf16 [N, S, 4*130] with a ones column per head pair."""
    n, s_, hd = values.shape
    v = _to_bf16(values).view(np.uint16)
    out = np.zeros((n, s_, 4, 130), np.uint16)
    out[..., 128] = np.uint16(0x3F80)  # 1.0 in bf16
    out[..., 0:128] = v.reshape(n, s_, 4, 128)
    return out.reshape(n, s_, 4 * 130).view(ml_dtypes.bfloat16)


def kernel(queries: np.ndarray, keys: np.ndarray, values: np.ndarray) -> np.ndarray:
    n, l_, h, d = queries.shape
    s_ = keys.shape[1]
    hd = h * d
    q = _to_bf16(np.asarray(queries, np.float32).reshape(n, l_, hd))
    k = _to_bf16(np.asarray(keys, np.float32).reshape(n, s_, hd))
    v = _pack_values(np.asarray(values, np.float32).reshape(n, s_, hd))
    nc = _cached_nc(l_, s_)
    in_maps = [{"queries": q[i], "keys": k[i], "values": v[i]} for i in range(n)]
    res = run_bass_kernel_spmd(nc, in_maps, core_ids=list(range(n)))
    out = np.empty((n, l_, h, d), np.float32)
    for i in range(n):
        out[i] = _from_bf16(res.results[i]["out"]).reshape(l_, h, d)
    return out


if __name__ == "__main__":
    nc = build_kernel()
    print("build ok")
